# revision 1
# baseline (speedup 1.0000x reference)
"""Trainium2 Bass kernel for the Covid compartment forecast model.

Computation (per posterior sample s):
    growth[t,s] = r_t[t]**(1/T_serial[s]) * delta[s]
    A[t,s]      = A[t-1,s] * growth[t,s]            (scan, A[-1] = warmup[-1])
    A_full      = concat(warmup, A)                 # [J+T, S]
    M[t,s]      = sum_j A_full[J-1-j+t, s] * rho[s] * pi[j, s]

Sharding: posterior-sample dimension S across 8 cores (pure data parallel).
On-chip layout: samples on partitions (tiles of 128), time on the free dim.

Engine plan per 128-sample tile:
  ACT : g = Exp(log_r * invT + ln_delta)   (per-partition scale/bias)
  DVE : A = tensor_tensor_scan(mult)       (the day recursion)
  FIR (32 taps) split across engines:
    PE   : diag(q_j) matmuls accumulated in PSUM
    ACT  : scaled copies B_j = A_shift * q_j, identity-matmul-accumulated by PE
    DVE  : scalar_tensor_tensor fused MACs into an SBUF accumulator
    Pool : scalar_tensor_tensor fused MACs into an SBUF accumulator
  PE   : 128x128 output transposes ([s,t] -> [t,s]), DMA from PSUM to DRAM.
"""

import numpy as np

import concourse.bacc as bacc
import concourse.bass as bass
import concourse.mybir as mybir
import concourse.tile as tile
from concourse.bass_utils import run_bass_kernel_spmd

F32 = mybir.dt.float32
F32R = mybir.dt.float32r
I32 = mybir.dt.int32
AF = mybir.ActivationFunctionType
OP = mybir.AluOpType

T = 1024
J = 32
S_TOTAL = 50000
NCORES = 8
P = 128
S_CORE = S_TOTAL // NCORES           # 6250
NTILES = (S_CORE + P - 1) // P       # 49
S_PAD = NTILES * P                   # 6272

# Tap assignment: which lag j is handled by which engine.
PE_TAPS = tuple(range(0, 8))
ACT_TAPS = tuple(range(8, 16))
POOL_TAPS = tuple(range(16, 22))
DVE_TAPS = tuple(range(22, 32))


def build(s_pad=S_PAD, pe_taps=PE_TAPS, act_taps=ACT_TAPS, dve_taps=DVE_TAPS,
          pool_taps=POOL_TAPS):
    assert s_pad % P == 0
    ntiles = s_pad // P
    taps = sorted(list(pe_taps) + list(act_taps) + list(dve_taps) + list(pool_taps))
    assert taps == list(range(J)), f"tap assignment must cover 0..{J-1}: {taps}"

    nc = bacc.Bacc("TRN2", target_bir_lowering=False, debug=False,
                   num_devices=NCORES)
    r = nc.dram_tensor("r_t", [1, T], F32, kind="ExternalInput").ap()
    wu = nc.dram_tensor("warmup", [J, s_pad], F32, kind="ExternalInput").ap()
    pi = nc.dram_tensor("pi", [J, s_pad], F32, kind="ExternalInput").ap()
    dl = nc.dram_tensor("delta", [1, s_pad], F32, kind="ExternalInput").ap()
    ts = nc.dram_tensor("t_serial", [1, s_pad], F32, kind="ExternalInput").ap()
    rh = nc.dram_tensor("rho", [1, s_pad], F32, kind="ExternalInput").ap()
    m = nc.dram_tensor("m_out", [T, s_pad], F32, kind="ExternalOutput").ap()

    # [1, s_pad] DRAM param -> [P, ntiles] SBUF layout: (p, i) = param[i*P + p]
    def param_ap(a):
        return bass.AP(tensor=a.tensor, offset=a.offset,
                       ap=[[1, P], [P, ntiles]])

    with tile.TileContext(nc) as tc:
        with (
            tc.tile_pool(name="singles", bufs=1) as singles,
            tc.tile_pool(name="loads", bufs=4) as loads,
            tc.tile_pool(name="apool", bufs=2) as apool,
            tc.tile_pool(name="gpool", bufs=2) as gpool,
            tc.tile_pool(name="qpool", bufs=4) as qpool,
            tc.tile_pool(name="diags", bufs=4) as diags,
            tc.tile_pool(name="bpool", bufs=3) as bpool,
            tc.tile_pool(name="mdp", bufs=2) as mdp,
            tc.tile_pool(name="msb", bufs=2) as msb,
            tc.tile_pool(name="mtp", bufs=2) as mtp,
            tc.tile_pool(name="mpsum", bufs=2, space="PSUM") as mpsum,
            tc.tile_pool(name="trpsum", bufs=4, space="PSUM") as trpsum,
        ):
            # ---- one-time setup ----
            iota_t = singles.tile([P, P], I32)
            nc.gpsimd.iota(iota_t, pattern=[[1, P]], base=0,
                           channel_multiplier=-1)
            ident = singles.tile([P, P], F32)
            nc.vector.tensor_scalar(out=ident, in0=iota_t, scalar1=0,
                                    scalar2=None, op0=OP.is_equal)
            ident_r = singles.tile([P, P], F32R)
            nc.vector.tensor_scalar(out=ident_r, in0=iota_t, scalar1=0,
                                    scalar2=None, op0=OP.is_equal)

            # broadcast r_t to all partitions (stride-0 DRAM read), then Ln
            r_bc = singles.tile([P, T], F32)
            nc.sync.dma_start(
                out=r_bc,
                in_=bass.AP(tensor=r.tensor, offset=r.offset,
                            ap=[[0, P], [1, T]]))
            lr_bc = singles.tile([P, T], F32)
            nc.scalar.activation(out=lr_bc, in_=r_bc, func=AF.Ln)

            dl_sb = singles.tile([P, ntiles], F32)
            nc.sync.dma_start(out=dl_sb, in_=param_ap(dl))
            ts_sb = singles.tile([P, ntiles], F32)
            nc.sync.dma_start(out=ts_sb, in_=param_ap(ts))
            rh_sb = singles.tile([P, ntiles], F32)
            nc.sync.dma_start(out=rh_sb, in_=param_ap(rh))

            # ---- per sample-tile ----
            for i in range(ntiles):
                s0 = i * P

                wu_nat = loads.tile([J, P], F32, tag="wu")
                nc.sync.dma_start(out=wu_nat, in_=wu[:, s0:s0 + P])
                pi_nat = loads.tile([J, P], F32, tag="pi")
                nc.sync.dma_start(out=pi_nat, in_=pi[:, s0:s0 + P])

                A_full = apool.tile([P, J + T], F32R)

                wuT = trpsum.tile([P, P], F32, tag="tr")
                nc.tensor.transpose(wuT[:, 0:J], wu_nat, ident[0:J, 0:J])
                nc.scalar.copy(out=A_full[:, 0:J], in_=wuT[:, 0:J])

                piT = trpsum.tile([P, P], F32, tag="tr")
                nc.tensor.transpose(piT[:, 0:J], pi_nat, ident[0:J, 0:J])
                q = qpool.tile([P, J], F32, tag="q")
                nc.vector.tensor_scalar(out=q, in0=piT[:, 0:J],
                                        scalar1=rh_sb[:, i:i + 1],
                                        scalar2=None, op0=OP.mult)

                invT = qpool.tile([P, 1], F32, tag="invT")
                nc.vector.reciprocal(out=invT, in_=ts_sb[:, i:i + 1])
                lnd = qpool.tile([P, 1], F32, tag="lnd")
                nc.scalar.activation(out=lnd, in_=dl_sb[:, i:i + 1], func=AF.Ln)

                g = gpool.tile([P, T], F32)
                nc.scalar.activation(out=g, in_=lr_bc, func=AF.Exp,
                                     bias=lnd, scale=invT)

                # A[t] = A[t-1] * g[t], A[-1] = warmup[:, J-1]
                nc.vector.tensor_tensor_scan(
                    out=A_full[:, J:J + T], data0=g, data1=g,
                    initial=A_full[:, J - 1:J], op0=OP.mult, op1=OP.bypass)

                # ---- FIR: M[t] = sum_j q[j] * A_full[31-j+t] ----
                # Only TensorE matmuls touch PSUM has_written, so every
                # contribution goes through the PE accumulation group.
                Mp = mpsum.tile([P, T], F32, tag="Mp")
                chunk_started = [False, False]

                def pe_acc(lhsT, rhs_base, stop=False):
                    for c in range(2):
                        lo = c * 512
                        nc.tensor.matmul(
                            Mp[:, lo:lo + 512], lhsT,
                            rhs_base[:, lo:lo + 512],
                            start=not chunk_started[c], stop=stop)
                        chunk_started[c] = True

                act_rest = act_taps
                for j in pe_taps:
                    dg = diags.tile([P, P], F32R, tag="diag")
                    nc.vector.tensor_scalar(out=dg, in0=ident,
                                            scalar1=q[:, j:j + 1],
                                            scalar2=None, op0=OP.mult)
                    pe_acc(dg, A_full[:, J - 1 - j:J - 1 - j + T])

                for j in act_rest:
                    B = bpool.tile([P, T], F32R, tag="b")
                    nc.scalar.activation(out=B,
                                         in_=A_full[:, J - 1 - j:J - 1 - j + T],
                                         func=AF.Copy, scale=q[:, j:j + 1])
                    pe_acc(ident_r, B)

                for j in pool_taps:
                    B = bpool.tile([P, T], F32R, tag="b")
                    nc.gpsimd.tensor_scalar(out=B,
                                            in0=A_full[:, J - 1 - j:J - 1 - j + T],
                                            scalar1=q[:, j:j + 1],
                                            scalar2=None, op0=OP.mult)
                    pe_acc(ident_r, B)

                Md = mdp.tile([P, T], F32R)
                for k, j in enumerate(dve_taps):
                    sh = A_full[:, J - 1 - j:J - 1 - j + T]
                    if k == 0:
                        nc.vector.tensor_scalar(out=Md, in0=sh,
                                                scalar1=q[:, j:j + 1],
                                                scalar2=None, op0=OP.mult)
                    else:
                        nc.vector.scalar_tensor_tensor(
                            out=Md, in0=sh, scalar=q[:, j:j + 1], in1=Md,
                            op0=OP.mult, op1=OP.add)
                pe_acc(ident_r, Md, stop=True)

                M_sb = msb.tile([P, T], F32)
                nc.scalar.copy(out=M_sb, in_=Mp)

                # transpose out: [s, t] tiles -> [t, s] DRAM
                # Mt_sb[:, b*P:(b+1)*P] holds M rows [b*P, (b+1)*P) of this
                # sample block; one strided DMA scatters all 8 blocks.
                Mt_sb = mtp.tile([P, T], F32)
                for b in range(T // P):
                    mt = trpsum.tile([P, P], F32, tag="tr")
                    nc.tensor.transpose(mt, M_sb[:, b * P:(b + 1) * P], ident)
                    nc.scalar.copy(out=Mt_sb[:, b * P:(b + 1) * P], in_=mt)
                out_ap = bass.AP(tensor=m.tensor, offset=m.offset + s0,
                                 ap=[[s_pad, P], [P * s_pad, T // P], [1, P]])
                nc.sync.dma_start(out=out_ap, in_=Mt_sb)

    nc.compile()
    return nc


_NC_CACHE = {}


def _get_nc():
    key = (S_PAD, PE_TAPS, ACT_TAPS, DVE_TAPS, POOL_TAPS)
    if key not in _NC_CACHE:
        _NC_CACHE[key] = build()
    return _NC_CACHE[key]


def _shard_inputs(r_t, warmup_A, delta, T_serial, rho_M, pi_M):
    """Slice the full inputs per core and pad S_CORE -> S_PAD."""
    r2 = np.ascontiguousarray(r_t, dtype=np.float32).reshape(1, T)
    in_maps = []
    for c in range(NCORES):
        lo, hi = c * S_CORE, (c + 1) * S_CORE
        pad = S_PAD - S_CORE

        def pad2(a, fill):
            a = np.asarray(a, dtype=np.float32)[:, lo:hi]
            return np.pad(a, ((0, 0), (0, pad)), constant_values=fill)

        def pad1(a, fill):
            a = np.asarray(a, dtype=np.float32)[lo:hi].reshape(1, -1)
            return np.pad(a, ((0, 0), (0, pad)), constant_values=fill)

        in_maps.append({
            "r_t": r2,
            "warmup": pad2(warmup_A, 1.0),
            "pi": pad2(pi_M, 1.0 / J),
            "delta": pad1(delta, 1.0),
            "t_serial": pad1(T_serial, 5.0),
            "rho": pad1(rho_M, 0.0),
        })
    return in_maps


def run(inputs, trace=False, **kwargs):
    """Run on 8 cores; returns (M [T, S_TOTAL] float32, BassKernelResults)."""
    nc = _get_nc()
    in_maps = _shard_inputs(**inputs)
    res = run_bass_kernel_spmd(nc, in_maps, core_ids=list(range(NCORES)),
                               trace=trace, **kwargs)
    M = np.concatenate(
        [res.results[c]["m_out"][:, :S_CORE] for c in range(NCORES)], axis=1)
    return np.ascontiguousarray(M, dtype=np.float32), res


def kernel(**inputs):
    M, _ = run(inputs)
    return M



# revision 2
# speedup vs baseline: 8.4804x; 8.4804x over previous
"""Trainium2 Bass kernel for the Covid compartment forecast model.

Computation (per posterior sample s):
    growth[t,s] = r_t[t]**(1/T_serial[s]) * delta[s]
    A[t,s]      = A[t-1,s] * growth[t,s]            (scan, A[-1] = warmup[-1])
    A_full      = concat(warmup, A)                 # [J+T, S]
    M[t,s]      = sum_j A_full[J-1-j+t, s] * rho[s] * pi[j, s]

Sharding: posterior-sample dimension S across 8 cores (pure data parallel).
On-chip layout: samples on partitions (tiles of 128), time on the free dim.
warmup/pi are pre-transposed on the host to [S, J] so no on-chip transpose
is needed for them; PSUM is fully dedicated to the two accumulator pools.

Engine plan per 128-sample tile:
  ACT : g = Exp(ln_r * invT + ln_delta); diag weights for most PE taps;
        scaled copies for a couple of tree taps; PSUM->SBUF output copy
  DVE : A = tensor_tensor_scan(mult) in fp32 state -> bf16 A_full;
        a few diag weights; scaled copies (tensor_scalar) for tree taps;
        pairwise-tree accumulation of the bf16 slab; final Mp+Md combine
  PE  : per-tap diag matmuls (bf16) accumulated in PSUM; output transposes
All tap streams are bf16 (fp32 scan state protects the recursion).
"""

import numpy as np

import concourse.bacc as bacc
import concourse.bass as bass
import concourse.mybir as mybir
import concourse.tile as tile
from concourse.bass_utils import run_bass_kernel_spmd

F32 = mybir.dt.float32
F32R = mybir.dt.float32r
BF16 = mybir.dt.bfloat16
I32 = mybir.dt.int32
AF = mybir.ActivationFunctionType
OP = mybir.AluOpType

T = 1024
J = 32
S_TOTAL = 50000
NCORES = 8
P = 128
S_CORE = S_TOTAL // NCORES           # 6250
NTILES = (S_CORE + P - 1) // P       # 49
S_PAD = NTILES * P                   # 6272

# Tap routing. PE taps go through diag-weight matmuls into PSUM; tree taps
# are scaled into a bf16 slab and pairwise-tree reduced on DVE.
N_PE = 24                 # taps 0..23 on PE
N_ACT_DIAG = 16           # of the PE taps, how many diags ACT builds
N_TREE = J - N_PE         # taps 24..31 via the slab tree
N_ACT_SCALE = 2           # of the tree taps, how many scaled copies ACT does


def build(n_pe=N_PE, n_act_diag=N_ACT_DIAG, n_act_scale=N_ACT_SCALE):
    n_tree = J - n_pe
    assert 1 <= n_pe <= J and 0 <= n_act_diag <= n_pe
    assert 0 <= n_act_scale <= n_tree

    nc = bacc.Bacc("TRN2", target_bir_lowering=False, debug=False,
                   num_devices=NCORES)
    r = nc.dram_tensor("r_t", [1, T], F32, kind="ExternalInput").ap()
    wu = nc.dram_tensor("warmup_t", [S_PAD, J], F32, kind="ExternalInput").ap()
    pi = nc.dram_tensor("pi_t", [S_PAD, J], F32, kind="ExternalInput").ap()
    dl = nc.dram_tensor("delta", [1, S_PAD], F32, kind="ExternalInput").ap()
    ts = nc.dram_tensor("t_serial", [1, S_PAD], F32, kind="ExternalInput").ap()
    rh = nc.dram_tensor("rho", [1, S_PAD], F32, kind="ExternalInput").ap()
    m = nc.dram_tensor("m_out", [T, S_PAD], F32, kind="ExternalOutput").ap()

    # [1, S_PAD] DRAM param -> [P, NTILES] SBUF layout: (p, i) = param[i*P + p]
    def param_ap(a):
        return bass.AP(tensor=a.tensor, offset=a.offset,
                       ap=[[1, P], [P, NTILES]])

    with tile.TileContext(nc) as tc:
        with (
            tc.tile_pool(name="singles", bufs=1) as singles,
            tc.tile_pool(name="loads", bufs=3) as loads,
            tc.tile_pool(name="apool", bufs=2) as apool,
            tc.tile_pool(name="gpool", bufs=2) as gpool,
            tc.tile_pool(name="qpool", bufs=3) as qpool,
            tc.tile_pool(name="diags", bufs=6) as diags,
            tc.tile_pool(name="slab", bufs=2) as slabp,
            tc.tile_pool(name="msb", bufs=2) as msb,
            tc.tile_pool(name="mtsb", bufs=2) as mtsb,
            tc.tile_pool(name="mpsum", bufs=2, space="PSUM") as mpsum,
            tc.tile_pool(name="mtpsum", bufs=2, space="PSUM") as mtpsum,
        ):
            # ---- one-time setup ----
            iota_t = singles.tile([P, P], I32)
            nc.gpsimd.iota(iota_t, pattern=[[1, P]], base=0,
                           channel_multiplier=-1)
            identb = singles.tile([P, P], BF16)
            nc.vector.tensor_scalar(out=identb, in0=iota_t, scalar1=0,
                                    scalar2=None, op0=OP.is_equal)
            ident_r = singles.tile([P, P], F32R)
            nc.vector.tensor_scalar(out=ident_r, in0=iota_t, scalar1=0,
                                    scalar2=None, op0=OP.is_equal)

            # broadcast r_t to all partitions (stride-0 DRAM read), then Ln
            r_bc = singles.tile([P, T], F32)
            nc.sync.dma_start(
                out=r_bc,
                in_=bass.AP(tensor=r.tensor, offset=r.offset,
                            ap=[[0, P], [1, T]]))
            lr_bc = singles.tile([P, T], F32)
            nc.scalar.activation(out=lr_bc, in_=r_bc, func=AF.Ln)

            dl_sb = singles.tile([P, NTILES], F32)
            nc.sync.dma_start(out=dl_sb, in_=param_ap(dl))
            ts_sb = singles.tile([P, NTILES], F32)
            nc.sync.dma_start(out=ts_sb, in_=param_ap(ts))
            rh_sb = singles.tile([P, NTILES], F32)
            nc.sync.dma_start(out=rh_sb, in_=param_ap(rh))

            # all per-tile scalars up front (keeps only Exp's ACT table
            # resident inside the loop)
            invT_all = singles.tile([P, NTILES], F32)
            nc.vector.reciprocal(out=invT_all, in_=ts_sb)
            lnd_all = singles.tile([P, NTILES], F32)
            nc.scalar.activation(out=lnd_all, in_=dl_sb, func=AF.Ln)

            # ---- per sample-tile ----
            for i in range(NTILES):
                s0 = i * P

                wuT = loads.tile([P, J], F32, tag="wu")
                nc.sync.dma_start(out=wuT, in_=wu[s0:s0 + P, :])
                piT = loads.tile([P, J], F32, tag="pi")
                nc.sync.dma_start(out=piT, in_=pi[s0:s0 + P, :])

                q = qpool.tile([P, J], F32, tag="q")
                nc.vector.tensor_scalar(out=q, in0=piT,
                                        scalar1=rh_sb[:, i:i + 1],
                                        scalar2=None, op0=OP.mult)

                A_full = apool.tile([P, J + T], BF16)
                nc.vector.tensor_copy(out=A_full[:, 0:J], in_=wuT)

                g = gpool.tile([P, T], F32)
                nc.scalar.activation(out=g, in_=lr_bc, func=AF.Exp,
                                     bias=lnd_all[:, i:i + 1],
                                     scale=invT_all[:, i:i + 1])

                # A[t] = A[t-1] * g[t]; fp32 internal state, bf16 output
                nc.vector.tensor_tensor_scan(
                    out=A_full[:, J:J + T], data0=g, data1=g,
                    initial=wuT[:, J - 1:J], op0=OP.mult, op1=OP.bypass)

                def shift(j):
                    return A_full[:, J - 1 - j:J - 1 - j + T]

                # ---- PE taps: diag(q_j) matmuls accumulated in PSUM ----
                Mp = mpsum.tile([P, T], F32, tag="Mp")
                for k in range(n_pe):
                    dg = diags.tile([P, P], BF16, tag="diag")
                    if k < n_act_diag:
                        nc.scalar.activation(out=dg, in_=ident_r,
                                             func=AF.Copy,
                                             scale=q[:, k:k + 1])
                    else:
                        nc.vector.tensor_scalar(out=dg, in0=identb,
                                                scalar1=q[:, k:k + 1],
                                                scalar2=None, op0=OP.mult)
                    last = k == n_pe - 1
                    for c in range(2):
                        lo = c * 512
                        nc.tensor.matmul(Mp[:, lo:lo + 512], dg,
                                         shift(k)[:, lo:lo + 512],
                                         start=(k == 0), stop=last)

                # ---- tree taps: scaled bf16 copies + pairwise reduce ----
                slab = slabp.tile([P, n_tree * T], BF16)
                for kk in range(n_tree):
                    j = n_pe + kk
                    dst = slab[:, kk * T:(kk + 1) * T]
                    if kk < n_act_scale:
                        nc.scalar.activation(out=dst, in_=shift(j),
                                             func=AF.Copy,
                                             scale=q[:, j:j + 1])
                    else:
                        nc.vector.tensor_scalar(out=dst, in0=shift(j),
                                                scalar1=q[:, j:j + 1],
                                                scalar2=None, op0=OP.mult)
                width = n_tree
                while width > 1:
                    half = width // 2
                    odd = width - 2 * half
                    nc.vector.tensor_tensor(
                        out=slab[:, 0:half * T],
                        in0=slab[:, 0:half * T],
                        in1=slab[:, half * T:2 * half * T],
                        op=OP.add)
                    if odd:
                        # fold the odd slab into slot 0 next round
                        nc.vector.tensor_tensor(
                            out=slab[:, 0:T],
                            in0=slab[:, 0:T],
                            in1=slab[:, (width - 1) * T:width * T],
                            op=OP.add)
                    width = half

                # ---- combine + output transpose ----
                M_sb = msb.tile([P, T], F32R)
                nc.vector.tensor_tensor(out=M_sb, in0=Mp,
                                        in1=slab[:, 0:T], op=OP.add)

                Mt_ps = mtpsum.tile([P, T], F32R, tag="mt")
                for b in range(T // P):
                    nc.tensor.transpose(Mt_ps[:, b * P:(b + 1) * P],
                                        M_sb[:, b * P:(b + 1) * P], ident_r)
                Mt_sb = mtsb.tile([P, T], F32)
                nc.scalar.copy(out=Mt_sb, in_=Mt_ps)

                # Mt_sb[:, b*P:(b+1)*P] holds M rows [b*P, (b+1)*P) of this
                # sample block; one strided DMA scatters all 8 blocks.
                out_ap = bass.AP(tensor=m.tensor, offset=m.offset + s0,
                                 ap=[[S_PAD, P], [P * S_PAD, T // P], [1, P]])
                nc.sync.dma_start(out=out_ap, in_=Mt_sb)

    nc.compile()
    return nc


_NC_CACHE = {}


def _get_nc(key=(N_PE, N_ACT_DIAG, N_ACT_SCALE)):
    if key not in _NC_CACHE:
        _NC_CACHE[key] = build(*key)
    return _NC_CACHE[key]


def _shard_inputs(r_t, warmup_A, delta, T_serial, rho_M, pi_M):
    """Slice the full inputs per core, pad S_CORE -> S_PAD, transpose
    warmup/pi to sample-major so the kernel needs no on-chip transpose."""
    r2 = np.ascontiguousarray(r_t, dtype=np.float32).reshape(1, T)
    wu_t = np.ascontiguousarray(np.asarray(warmup_A, dtype=np.float32).T)
    pi_t = np.ascontiguousarray(np.asarray(pi_M, dtype=np.float32).T)
    in_maps = []
    for c in range(NCORES):
        lo, hi = c * S_CORE, (c + 1) * S_CORE
        pad = S_PAD - S_CORE

        def pad2t(a, fill):
            return np.pad(a[lo:hi], ((0, pad), (0, 0)), constant_values=fill)

        def pad1(a, fill):
            a = np.asarray(a, dtype=np.float32)[lo:hi].reshape(1, -1)
            return np.pad(a, ((0, 0), (0, pad)), constant_values=fill)

        in_maps.append({
            "r_t": r2,
            "warmup_t": pad2t(wu_t, 1.0),
            "pi_t": pad2t(pi_t, 1.0 / J),
            "delta": pad1(delta, 1.0),
            "t_serial": pad1(T_serial, 5.0),
            "rho": pad1(rho_M, 0.0),
        })
    return in_maps


def run(inputs, trace=False, key=(N_PE, N_ACT_DIAG, N_ACT_SCALE), **kwargs):
    """Run on 8 cores; returns (M [T, S_TOTAL] float32, BassKernelResults)."""
    nc = _get_nc(key)
    in_maps = _shard_inputs(**inputs)
    res = run_bass_kernel_spmd(nc, in_maps, core_ids=list(range(NCORES)),
                               trace=trace, **kwargs)
    M = np.concatenate(
        [res.results[c]["m_out"][:, :S_CORE] for c in range(NCORES)], axis=1)
    return np.ascontiguousarray(M, dtype=np.float32), res


def kernel(**inputs):
    M, _ = run(inputs)
    return M
